# revision 45
# baseline (speedup 1.0000x reference)
"""Bahdanau additive-attention kernel for Trainium2, data-parallel over
batch across 8 NeuronCores.

Per batch b:
    energy  = tanh(dec_proj[b] + enc[b] @ W_enc + b_score)   # (L, DEC)
    scores  = energy @ v                                     # (L,)
    alpha   = softmax(scores)
    att[b]  = alpha @ enc[b]                                 # (2E,)

On-device layout (per core, 8 batches):
  - enc is staged host-side TWICE, both partition-major-tiled so DMA
    descriptors are 4-16KB runs: enc8_t (fp8e4m3, chunk-major) feeds the
    PE energy matmul in perf_mode=DoubleRow (two k-subtiles per
    instruction, measured ~2x over bf16/f32r at N=512); encb_t (bf16,
    full-L tiles) feeds the DVE attention reduce. W_enc is pre-scaled by
    128 on the host so its values sit mid-range in e4m3; the tanh
    activation rescales the psum by 1/128.
  - dec_proj preamble runs in bf16 (error negligible vs fp8 energy).
  - scores = v . energyT via PE matvec in bf16 over d-tiles.
  - softmax skips the max-subtraction: |scores| <= sum|v| = 32, safely
    inside the fp32 exp range. Raw scores broadcast to 128 partitions
    with a K=1 ones-matmul; Exp runs on the broadcast tile (bf16 out)
    with accum_out giving the replicated denominator per partition.
  - att^T accumulates via fused scalar_tensor_tensor on DVE:
    accum_out[e,1] = sum_l enc_bf16[e,l] * exp_scores[l], one full-L
    instruction per (batch, k-tile) to amortize DVE fixed overhead. The
    last batch runs per-chunk so only one chunk's reduce sits in the
    kernel tail.
  - startup is DMA-paced, so batch 0 chunk 0 consumes enc in half-tile
    arrival order with the dec_proj preamble matmuls behind it.
"""

import numpy as np
import ml_dtypes
from contextlib import ExitStack

import concourse.bass as bass
import concourse.tile as tile
from concourse import mybir
from concourse.bass_utils import run_bass_kernel_spmd
from concourse.vector_clock import ScopedClock, VectorClock

N_CORES = 8
B, L, DEC, ENC2 = 64, 1024, 1024, 2048
BL = B // N_CORES  # batches per core
KT = ENC2 // 128   # contraction tiles over e
KP = KT // 2       # DoubleRow pairs over e
KH = KT // 2       # k-tiles per half (SBUF tile granularity)
DT = DEC // 128    # d tiles
LC = 512           # l-chunk (one PSUM bank of f32)
NLC = L // LC
WSCALE = 128.0     # host-side W_enc scaling for fp8 range
VSCALE = 32.0      # host-side v scaling for fp8 range

F32 = mybir.dt.float32
F32R = mybir.dt.float32r
F8 = mybir.dt.float8e4
BF16 = mybir.dt.bfloat16
AF = mybir.ActivationFunctionType
ALU = mybir.AluOpType
PM = mybir.MatmulPerfMode


def _patch_tile_drain():
    """Workarounds for this container's walrus build.

    1. The Tile tail drain carries one sem wait per touched proc; walrus
       rejects >2 on the CTRL encoding. Split the waits onto single-wait
       SP nops (SP executes in order, so the drain then needs none).
    2. Any instruction with 2+ sem waits can fail codegen (the matmul
       LW encoding holds a single wait). Split multi-wait instructions:
       excess waits move onto same-engine InstNoOp carriers inserted
       just before; engine program order makes this equivalent.
    """
    if getattr(tile.TileContext, "_drain_patched", False):
        return

    def _drain_and_barrier(self, tick_clock, wait_clock):
        vec = list(tick_clock.global_clock)
        n = len(vec)
        for i in range(n):
            if vec[i] <= 0:
                continue
            part = [0] * n
            part[i] = vec[i]
            nop_inst = self.nc.sync.nop(nofuse=True)
            wait_clock.add_sem_waits(
                nop_inst.ins, ScopedClock({None: VectorClock(part)})
            )
        self.nc.sync.drain()
        self.nc.all_engine_barrier()
        assert self.sems is not None
        popped = self.nc._tile_sem_poison_stack.pop()
        assert popped is self._sem_poison
        self.nc.clear_and_free_semaphores(list(self.sems.allocated().values()))
        self.nc.all_engine_barrier()

    tile.TileContext._drain_and_barrier = _drain_and_barrier

    import bass_rust

    orig_lower = tile.TileContext._lower_ordered_insts

    def _lower_with_wait_split(self, ordered):
        for insts in ordered.values():
            expanded = []
            for inst in insts:
                si = inst.sync_info
                waits = list(si.on_wait) if si and si.on_wait else []
                if len(waits) > 1:
                    for w in waits[:-1]:
                        nop = mybir.InstNoOp(
                            name=self.nc.get_next_instruction_name(),
                            engine=inst.engine,
                            bass_nofuse=True,
                            sync_info=bass_rust.SyncInfo(on_wait=[w], on_update=[]),
                        )
                        self.nc.register_instruction(nop)
                        expanded.append(nop)
                    inst.sync_info = bass_rust.SyncInfo(
                        on_wait=[waits[-1]],
                        on_update=list(si.on_update) if si.on_update else [],
                    )
                expanded.append(inst)
            insts[:] = expanded
        return orig_lower(self, ordered)

    tile.TileContext._lower_ordered_insts = _lower_with_wait_split
    tile.TileContext._drain_patched = True


def build_nc():
    _patch_tile_drain()
    nc = bass.Bass()
    # partition-major tiled layouts (see shard_inputs)
    enc8_t = nc.declare_dram_parameter(
        "enc8_t", [BL, NLC, 2, 128, KH, LC], F8, isOutput=False
    )
    encb_t = nc.declare_dram_parameter(
        "encb_t", [BL, 2, 128, KH, L], BF16, isOutput=False
    )
    wenc8_d = nc.declare_dram_parameter(
        "wenc8", [128, KT, DEC], F8, isOutput=False
    )
    bias_d = nc.declare_dram_parameter("bias_kpb", [128, DT, BL], F32, isOutput=False)
    v_mat = nc.declare_dram_parameter("v_mat", [128, DT, 16], F8, isOutput=False)
    eye = nc.declare_dram_parameter("eye", [128, 128], F32, isOutput=False)
    ones = nc.declare_dram_parameter("ones", [1, 128], BF16, isOutput=False)
    att = nc.declare_dram_parameter("att", [BL, ENC2], F32, isOutput=True)

    with tile.TileContext(nc) as tc, ExitStack() as ctx:
        singles = ctx.enter_context(tc.tile_pool(name="singles", bufs=1))
        smalls = ctx.enter_context(tc.tile_pool(name="smalls", bufs=2))
        enc8_pool = ctx.enter_context(tc.tile_pool(name="enc8", bufs=8))
        encb_pool = ctx.enter_context(tc.tile_pool(name="encb", bufs=6))
        energy_pool = ctx.enter_context(tc.tile_pool(name="energy", bufs=3))
        wexp_pool = ctx.enter_context(tc.tile_pool(name="wexp", bufs=2))
        prod_pool = ctx.enter_context(tc.tile_pool(name="prod", bufs=2))
        ep_ps = ctx.enter_context(tc.tile_pool(name="ep_ps", bufs=3, space="PSUM"))
        sc_ps = ctx.enter_context(tc.tile_pool(name="sc_ps", bufs=2, space="PSUM"))
        wb_ps = ctx.enter_context(tc.tile_pool(name="wb_ps", bufs=2, space="PSUM"))
        att_ps_pool = ctx.enter_context(
            tc.tile_pool(name="att_ps", bufs=1, space="PSUM")
        )

        # ---- persistent tiles -------------------------------------------
        wenc = singles.tile([128, KT, DEC], F8)  # W_enc*128, (e-tile, k) x d
        v_sb = singles.tile([128, DT, 16], F8)
        eye_sb = singles.tile([128, 128], F32)
        bias_sb = singles.tile([128, DT, BL], F32)  # dec_proj + b_score
        att_all = singles.tile([128, KT * BL], F32)  # att^T cols = b*KT+k
        ones_sb = singles.tile([1, 128], BF16)

        def alloc_chunk8(nm):
            a = enc8_pool.tile([128, KH, LC], F8, tag="enc", name=f"{nm}a")
            bb = enc8_pool.tile([128, KH, LC], F8, tag="enc", name=f"{nm}b")
            return (a, bb)

        def load_chunk8(b, c, ch):
            for h in range(2):
                nc.sync.dma_start(out=ch[h], in_=enc8_t[b, c, h])

        def alloc_encb(nm):
            a = encb_pool.tile([128, KH, L], BF16, tag="encb", name=f"{nm}a")
            bb = encb_pool.tile([128, KH, L], BF16, tag="encb", name=f"{nm}b")
            return (a, bb)

        def load_encb_parts(b, ch, parts):
            for part in parts:
                h, q = part // 2, (part % 2) * (KH // 2)
                nc.sync.dma_start(
                    out=ch[h][:, q : q + KH // 2, :],
                    in_=encb_t[b, h][:, q : q + KH // 2, :],
                )

        def enc_pair(ch, kp):
            """[128, 2, LC] DoubleRow rhs slice for pair (2kp, 2kp+1)."""
            k = 2 * kp
            t, kk = (ch[0], k) if k < KH else (ch[1], k - KH)
            return t[:, kk : kk + 2, :]

        def encb_sl(ch, k, lo=0, width=L):
            t, kk = (ch[0], k) if k < KH else (ch[1], k - KH)
            return t[:, kk, lo : lo + width]

        def w_pair(kp, dt):
            """[128, 2, 128] DoubleRow lhsT slice."""
            return wenc[:, 2 * kp : 2 * kp + 2, dt * 128 : (dt + 1) * 128]

        # ---- startup DMA, in data-arrival order -------------------------
        enc00 = alloc_chunk8("enc00")
        nc.sync.dma_start(out=enc00[0][:, 0:2, :], in_=enc8_t[0, 0, 0][:, 0:2, :])
        nc.sync.dma_start(out=wenc[:, 0:2, :], in_=wenc8_d[:, 0:2, :])
        nc.sync.dma_start(out=enc00[0][:, 2:4, :], in_=enc8_t[0, 0, 0][:, 2:4, :])
        nc.sync.dma_start(out=wenc[:, 2:4, :], in_=wenc8_d[:, 2:4, :])
        nc.sync.dma_start(out=enc00[0][:, 4:8, :], in_=enc8_t[0, 0, 0][:, 4:8, :])
        nc.sync.dma_start(out=wenc[:, 4:8, :], in_=wenc8_d[:, 4:8, :])
        nc.sync.dma_start(out=bias_sb, in_=bias_d[:, :, :])
        nc.sync.dma_start(out=v_sb, in_=v_mat[:, :, :])
        nc.sync.dma_start(out=enc00[1], in_=enc8_t[0, 0, 1])
        nc.sync.dma_start(out=wenc[:, 8:12, :], in_=wenc8_d[:, 8:12, :])
        nc.sync.dma_start(out=wenc[:, 12:16, :], in_=wenc8_d[:, 12:16, :])
        nc.sync.dma_start(out=eye_sb, in_=eye[:, :])
        nc.sync.dma_start(out=ones_sb, in_=ones[:, :])

        # ---- chunk (0,0): consume pairs in half-arrival order ----------
        ps00 = {
            dt: ep_ps.tile([128, LC], F32, tag="ep", name=f"ps00_{dt}")
            for dt in range(3)
        }
        for kp in range(KP):
            for dt in range(3):
                nc.tensor.matmul(
                    ps00[dt],
                    lhsT=w_pair(kp, dt),
                    rhs=enc_pair(enc00, kp),
                    start=(kp == 0),
                    stop=(kp == KP - 1),
                    perf_mode=PM.DoubleRow,
                )

        def tanh_pair(ps, energy, b, dt):
            """tanh one d-tile's psum into half of an fp8 pair tile."""
            nc.scalar.activation(
                out=energy[:, dt % 2, :],
                in_=ps,
                func=AF.Tanh,
                bias=bias_sb[:, dt, b : b + 1],
                scale=1.0 / WSCALE,
            )

        def score_pair(sc, energy, dtp):
            """DoubleRow matvec over one d-tile pair: sc += v . energy."""
            nc.tensor.matmul(
                sc,
                lhsT=v_sb[:, 2 * dtp : 2 * dtp + 2, 0:1],
                rhs=energy,
                start=(dtp == 0),
                stop=(dtp == DT // 2 - 1),
                perf_mode=PM.DoubleRow,
            )

        def alloc_energy(nm):
            return energy_pool.tile([128, 2, LC], F8, tag="energy", name=nm)

        sc00 = sc_ps.tile([1, LC], F32, tag="sc")
        en00 = {}
        for dtp in range(DT // 2):
            en00[dtp] = None
        cur_en = alloc_energy("en00_0")
        tanh_pair(ps00[0], cur_en, 0, 0)
        tanh_pair(ps00[1], cur_en, 0, 1)
        score_pair(sc00, cur_en, 0)
        cur_en = alloc_energy("en00_1")
        tanh_pair(ps00[2], cur_en, 0, 2)
        for dt in range(3, DT):
            ps = ep_ps.tile([128, LC], F32, tag="ep", name=f"ps00b_{dt}")
            for kp in range(KP):
                nc.tensor.matmul(
                    ps,
                    lhsT=w_pair(kp, dt),
                    rhs=enc_pair(enc00, kp),
                    start=(kp == 0),
                    stop=(kp == KP - 1),
                    perf_mode=PM.DoubleRow,
                )
            if dt % 2 == 0:
                cur_en = alloc_energy(f"en00_{dt // 2}")
            tanh_pair(ps, cur_en, 0, dt)
            if dt % 2 == 1:
                score_pair(sc00, cur_en, dt // 2)

        def make_chunk(b, c, enc_tile):
            """Return (sc, group emitters, matvec emitters) for one
            chunk. The caller interleaves them with the previous chunk's
            tail ops (software pipelining) so ACT latencies never sit
            exposed in the in-order PE stream."""
            sc = sc_ps.tile([1, LC], F32, tag="sc", name=f"sc_{b}_{c}")
            ens = {}

            def make_group(dtp):
                def emit():
                    en = alloc_energy(f"en_{b}_{c}_{dtp}")
                    for i in range(2):
                        dt = 2 * dtp + i
                        ps = ep_ps.tile(
                            [128, LC], F32, tag="ep", name=f"dm_{b}_{c}_{dt}"
                        )
                        for kp in range(KP):
                            nc.tensor.matmul(
                                ps,
                                lhsT=w_pair(kp, dt),
                                rhs=enc_pair(enc_tile, kp),
                                start=(kp == 0),
                                stop=(kp == KP - 1),
                                perf_mode=PM.DoubleRow,
                            )
                        tanh_pair(ps, en, b, dt)
                    ens[dtp] = en

                return emit

            def make_mv(dtp):
                def emit():
                    score_pair(sc, ens[dtp], dtp)

                return emit

            groups = [make_group(p) for p in range(DT // 2)]
            mvs = [make_mv(p) for p in range(DT // 2)]
            return sc, groups, mvs

        def score_to_wexp(sc, wexp, b, c):
            """Exp the raw chunk scores into the batch's broadcast weight
            tile; returns the per-partition denominator contribution."""
            s_sb = smalls.tile([1, LC], BF16, tag="ssb", name=f"ssb_{b}_{c}")
            nc.scalar.copy(out=s_sb, in_=sc)
            wb = wb_ps.tile([128, LC], F32, tag="wb", name=f"wb_{b}_{c}")
            nc.tensor.matmul(wb, lhsT=ones_sb, rhs=s_sb, start=True, stop=True)
            den_c = smalls.tile([128, 1], F32, tag=f"den{c}", name=f"den_{b}_{c}")
            nc.scalar.activation(
                out=wexp[:, c * LC : (c + 1) * LC],
                in_=wb,
                func=AF.Exp,
                bias=0.0,
                scale=1.0 / VSCALE,
                accum_out=den_c,
            )
            return den_c

        def batch_att(b, encb_tile, wexp):
            """Fused weighted reduce over the full L per k-tile, split
            ~3:1 between DVE (fused STT) and ACT (accum-copy over DVE
            pair-products). Emits the DVE work now; returns a closure
            with the ACT half so the caller can defer it past the next
            chunk's tanh chain (else ACT bunches and PE starves on psum
            banks)."""
            w_pairbc = bass.AP(
                tensor=wexp.tensor,
                offset=wexp.offset,
                ap=[wexp.ap[0], [0, 2], wexp.ap[1]],
            )
            for k in range(KT):
                col = b * KT + k
                prod = prod_pool.tile([128, L], BF16, tag="prod", name=f"pr_{b}_{k}")
                nc.vector.scalar_tensor_tensor(
                    out=prod,
                    in0=encb_sl(encb_tile, k),
                    scalar=1.0,
                    in1=wexp,
                    op0=ALU.mult,
                    op1=ALU.mult,
                    accum_out=att_all[:, col : col + 1],
                )

            def act_half():
                pass

            return act_half

        def chunk_att_tail(b, c, encb_tile, wexp, n_act=4):
            """Kernel-tail variant: split the reduce between DVE (fused
            STT) and ACT (accum-copy over DVE pair-products, last n_act
            k-tiles) so the exposed tail shortens. Emitted inline: the
            ACT half overlaps the next chunk's window via queue slack."""
            atmp = None
            if c > 0:
                atmp = smalls.tile([128, KT], F32, tag="atmp", name=f"atmpt_{b}_{c}")

            def dst(k):
                if c == 0:
                    col = b * KT + k
                    return att_all[:, col : col + 1]
                return atmp[:, k : k + 1]

            w_sl = wexp[:, c * LC : (c + 1) * LC]
            w_pairbc = bass.AP(
                tensor=w_sl.tensor,
                offset=w_sl.offset,
                ap=[w_sl.ap[0], [0, 2], w_sl.ap[1]],
            )
            pprods = []
            for kp in range(KT // 2 - n_act // 2, KT // 2):
                k = 2 * kp
                t, kk = (encb_tile[0], k) if k < KH else (encb_tile[1], k - KH)
                prod = prod_pool.tile(
                    [128, 2, LC], BF16, tag="prodt", name=f"prp_{b}_{c}_{kp}"
                )
                nc.vector.tensor_mul(
                    out=prod,
                    in0=t[:, kk : kk + 2, c * LC : (c + 1) * LC],
                    in1=w_pairbc,
                )
                pprods.append((k, prod))
            for k in range(KT - n_act):
                prod = prod_pool.tile(
                    [128, L], BF16, tag="prod", name=f"prt_{b}_{c}_{k}"
                )
                nc.vector.scalar_tensor_tensor(
                    out=prod[:, 0:LC],
                    in0=encb_sl(encb_tile, k, c * LC, LC),
                    scalar=1.0,
                    in1=w_sl,
                    op0=ALU.mult,
                    op1=ALU.mult,
                    accum_out=dst(k),
                )

            scr = smalls.tile([128, LC], BF16, tag="ascr", name=f"ascr_{b}_{c}")
            for k, prod in pprods:
                for i in range(2):
                    nc.scalar.activation(
                        out=scr,
                        in_=prod[:, i, :],
                        func=AF.Copy,
                        bias=0.0,
                        scale=1.0,
                        accum_out=dst(k + i),
                    )
            if c > 0:
                cols = slice(b * KT, (b + 1) * KT)
                nc.vector.tensor_add(
                    out=att_all[:, cols], in0=att_all[:, cols], in1=atmp
                )

        def batch_epilogue(b, dens):
            """Transpose the raw attention columns and store, folding the
            softmax normalization into the ACT psum->sbuf copy (scale)."""
            rden = smalls.tile([128, 1], F32, tag="rden")
            nc.vector.tensor_add(out=rden, in0=dens[0], in1=dens[1])
            for extra in dens[2:]:
                nc.vector.tensor_add(out=rden, in0=rden, in1=extra)
            nc.vector.reciprocal(out=rden, in_=rden)
            cols = slice(b * KT, (b + 1) * KT)
            att_bt = att_ps_pool.tile([KT, 128], F32, tag="abt")
            nc.tensor.transpose(att_bt, att_all[:, cols], eye_sb)
            att_sb = smalls.tile([KT, 128], F32, tag="asb")
            nc.scalar.activation(
                out=att_sb,
                in_=att_bt,
                func=AF.Copy,
                bias=0.0,
                scale=rden[0:KT, :],
            )
            nc.sync.dma_start(
                out=att[b].rearrange("(k p) -> k p", p=128), in_=att_sb
            )

        # ---- main loop: 1-chunk software pipeline -----------------------
        # Emission per iteration: [g0, prev.mv3, g1, prev bookkeeping
        # (scores->exp, reduces, epilogues), mv0, g2, mv1, g3, mv2] so
        # the previous chunk's ACT-latency tail hides behind this
        # chunk's matmul groups in the in-order PE stream.
        wexp_map = {}
        encb_map = {}
        state = {"dens": [], "prev_dens": None, "pending": []}

        def bookkeeping(pb, pc, sc):
            if pc == 0:
                wexp_map[pb] = wexp_pool.tile(
                    [128, L], BF16, tag="wexp", name=f"wexp_{pb}"
                )
                state["dens"] = []
            wexp = wexp_map[pb]
            state["dens"].append(score_to_wexp(sc, wexp, pb, pc))
            for fn in state["pending"]:
                fn()
            state["pending"] = []
            if pc == 1 and pb > 0:
                batch_epilogue(pb - 1, state["prev_dens"])
            if pb >= BL - 2:
                n_act = 6 if (pb == BL - 1 and pc == 1) else 4
                chunk_att_tail(pb, pc, encb_map[pb], wexp, n_act=n_act)
            elif pc == 1:
                state["pending"].append(batch_att(pb, encb_map[pb], wexp))
            if pc == 1:
                state["prev_dens"] = state["dens"]

        prev_bk = (0, 0, sc00)
        prev_mv3 = None
        for b in range(BL):
            for c in range(NLC):
                if (b, c) == (0, 0):
                    continue
                enc_tile = alloc_chunk8(f"enc_{b}_{c}")
                load_chunk8(b, c, enc_tile)
                # bf16-copy prefetch: ~2MB per chunk slot, pulled in
                # for the last two batches (their per-chunk tails read
                # all k-tiles one chunk earlier than batch-wide reduces)
                def _encb(bb, parts, alloc=False):
                    if alloc:
                        encb_map[bb] = alloc_encb(f"encb_{bb}")
                    load_encb_parts(bb, encb_map[bb], parts)

                sched = {
                    (0, 1): [(0, [0, 1], True)],
                    (1, 0): [(0, [2, 3]), (1, [0], True)],
                    (1, 1): [(1, [1, 2])],
                    (2, 0): [(1, [3]), (2, [0], True)],
                    (2, 1): [(2, [1, 2])],
                    (3, 0): [(2, [3]), (3, [0], True)],
                    (3, 1): [(3, [1, 2])],
                    (4, 0): [(3, [3]), (4, [0], True)],
                    (4, 1): [(4, [1, 2])],
                    (5, 0): [(4, [3]), (5, [0, 1], True)],
                    (5, 1): [(5, [2, 3])],
                    (6, 0): [(6, [0, 1], True)],
                    (6, 1): [(6, [2, 3]), (7, [0], True)],
                    (7, 0): [(7, [1, 2, 3])],
                }
                for item in sched.get((b, c), []):
                    _encb(item[0], item[1], len(item) > 2 and item[2])
                sc, groups, mvs = make_chunk(b, c, enc_tile)
                groups[0]()
                if prev_mv3 is not None:
                    prev_mv3()
                groups[1]()
                bookkeeping(*prev_bk)
                mvs[0]()
                groups[2]()
                mvs[1]()
                groups[3]()
                mvs[2]()
                prev_bk = (b, c, sc)
                prev_mv3 = mvs[3]
        prev_mv3()
        bookkeeping(*prev_bk)
        for fn in state["pending"]:
            fn()
        batch_epilogue(BL - 1, state["prev_dens"])

    return nc


def shard_inputs(dec_hidden, enc_output, W_score, b_score, v):
    """Full inputs -> per-core input maps (host-side layout staging)."""
    dec_hidden = np.ascontiguousarray(dec_hidden, dtype=np.float32)
    W_score = np.asarray(W_score, dtype=np.float32)
    # dec_proj + b_score computed host-side (0.05% of the FLOPs)
    bias_full = dec_hidden @ W_score[:DEC] + np.asarray(b_score, dtype=np.float32)
    # W_enc tiled partition-major: [p, k, d]
    wenc8 = np.ascontiguousarray(
        (W_score[DEC:] * WSCALE).reshape(KT, 128, DEC).transpose(1, 0, 2)
    ).astype(ml_dtypes.float8_e4m3)
    v_pd = np.asarray(v, dtype=np.float32).reshape(DT, 128).T * VSCALE
    v_mat = np.zeros((128, DT, 16), dtype=ml_dtypes.float8_e4m3)
    v_mat[:, :, 0] = v_pd.astype(ml_dtypes.float8_e4m3)
    eye = np.eye(128, dtype=np.float32)

    in_maps = []
    for core in range(N_CORES):
        sl = slice(core * BL, (core + 1) * BL)
        # (L, BL, 2E) -> (BL, 2E, L)
        enc_t = np.ascontiguousarray(
            np.asarray(enc_output[:, sl, :], dtype=np.float32).transpose(1, 2, 0)
        )
        # fp8 chunk-major partition-tiled: [b, c, half, p, k, l]
        enc8_t = np.ascontiguousarray(
            enc_t.reshape(BL, 2, KH, 128, NLC, LC).transpose(0, 4, 1, 3, 2, 5)
        ).astype(ml_dtypes.float8_e4m3)
        # bf16 full-L partition-tiled: [b, half, p, k, l]
        encb_t = np.ascontiguousarray(
            enc_t.reshape(BL, 2, KH, 128, L).transpose(0, 1, 3, 2, 4)
        ).astype(ml_dtypes.bfloat16)
        # (BL, DEC) -> [p, dt, b]
        bias_kpb = np.ascontiguousarray(
            bias_full[sl].T.reshape(DT, 128, BL).transpose(1, 0, 2)
        )
        in_maps.append(
            {
                "enc8_t": enc8_t,
                "encb_t": encb_t,
                "ones": np.ones((1, 128), dtype=ml_dtypes.bfloat16),
                "bias_kpb": bias_kpb,
                "wenc8": wenc8,
                "v_mat": v_mat,
                "eye": eye,
            }
        )
    return in_maps


_NC_CACHE = None


def kernel(dec_hidden, enc_output, W_score, b_score, v):
    global _NC_CACHE
    if _NC_CACHE is None:
        _NC_CACHE = build_nc()
    nc = _NC_CACHE
    in_maps = shard_inputs(dec_hidden, enc_output, W_score, b_score, v)
    res = run_bass_kernel_spmd(nc, in_maps, list(range(N_CORES)))
    return np.concatenate([res.results[i]["att"] for i in range(N_CORES)], axis=0)


# revision 46
# speedup vs baseline: 1.1919x; 1.1919x over previous
"""Bahdanau additive-attention kernel for Trainium2, data-parallel over
batch across 8 NeuronCores.

Per batch b:
    energy  = tanh(dec_proj[b] + enc[b] @ W_enc + b_score)   # (L, DEC)
    scores  = energy @ v                                     # (L,)
    alpha   = softmax(scores)
    att[b]  = alpha @ enc[b]                                 # (2E,)

On-device layout (per core, 8 batches):
  - enc is staged host-side TWICE, both partition-major-tiled so DMA
    descriptors are 4-16KB runs: enc8_t (fp8e4m3, chunk-major) feeds the
    PE energy matmul in perf_mode=DoubleRow (two k-subtiles per
    instruction, measured ~2x over bf16/f32r at N=512); encb_t (bf16,
    full-L tiles) feeds the DVE attention reduce. W_enc is pre-scaled by
    128 on the host so its values sit mid-range in e4m3; the tanh
    activation rescales the psum by 1/128.
  - dec_proj preamble runs in bf16 (error negligible vs fp8 energy).
  - scores = v . energyT via PE matvec in bf16 over d-tiles.
  - softmax skips the max-subtraction: |scores| <= sum|v| = 32, safely
    inside the fp32 exp range. Raw scores broadcast to 128 partitions
    with a K=1 ones-matmul; Exp runs on the broadcast tile (bf16 out)
    with accum_out giving the replicated denominator per partition.
  - att^T accumulates via fused scalar_tensor_tensor on DVE:
    accum_out[e,1] = sum_l enc_bf16[e,l] * exp_scores[l], one full-L
    instruction per (batch, k-tile) to amortize DVE fixed overhead. The
    last batch runs per-chunk so only one chunk's reduce sits in the
    kernel tail.
  - startup is DMA-paced, so batch 0 chunk 0 consumes enc in half-tile
    arrival order with the dec_proj preamble matmuls behind it.
"""

import numpy as np
import ml_dtypes
from contextlib import ExitStack

import concourse.bass as bass
import concourse.tile as tile
from concourse import mybir
from concourse.bass_utils import run_bass_kernel_spmd
from concourse.vector_clock import ScopedClock, VectorClock

N_CORES = 8
B, L, DEC, ENC2 = 64, 1024, 1024, 2048
BL = B // N_CORES  # batches per core
KT = ENC2 // 128   # contraction tiles over e
KP = KT // 2       # DoubleRow pairs over e
KH = KT // 2       # k-tiles per half (SBUF tile granularity)
DT = DEC // 128    # d tiles
LC = 512           # l-chunk (one PSUM bank of f32)
NLC = L // LC
WSCALE = 128.0     # host-side W_enc scaling for fp8 range
VSCALE = 32.0      # host-side v scaling for fp8 range

F32 = mybir.dt.float32
F32R = mybir.dt.float32r
F8 = mybir.dt.float8e4
BF16 = mybir.dt.bfloat16
AF = mybir.ActivationFunctionType
ALU = mybir.AluOpType
PM = mybir.MatmulPerfMode


def _patch_tile_drain():
    """Workarounds for this container's walrus build.

    1. The Tile tail drain carries one sem wait per touched proc; walrus
       rejects >2 on the CTRL encoding. Split the waits onto single-wait
       SP nops (SP executes in order, so the drain then needs none).
    2. Any instruction with 2+ sem waits can fail codegen (the matmul
       LW encoding holds a single wait). Split multi-wait instructions:
       excess waits move onto same-engine InstNoOp carriers inserted
       just before; engine program order makes this equivalent.
    """
    if getattr(tile.TileContext, "_drain_patched", False):
        return

    def _drain_and_barrier(self, tick_clock, wait_clock):
        vec = list(tick_clock.global_clock)
        n = len(vec)
        for i in range(n):
            if vec[i] <= 0:
                continue
            part = [0] * n
            part[i] = vec[i]
            nop_inst = self.nc.sync.nop(nofuse=True)
            wait_clock.add_sem_waits(
                nop_inst.ins, ScopedClock({None: VectorClock(part)})
            )
        self.nc.sync.drain()
        self.nc.all_engine_barrier()
        assert self.sems is not None
        popped = self.nc._tile_sem_poison_stack.pop()
        assert popped is self._sem_poison
        self.nc.clear_and_free_semaphores(list(self.sems.allocated().values()))
        self.nc.all_engine_barrier()

    tile.TileContext._drain_and_barrier = _drain_and_barrier

    import bass_rust

    orig_lower = tile.TileContext._lower_ordered_insts

    def _lower_with_wait_split(self, ordered):
        for insts in ordered.values():
            expanded = []
            for inst in insts:
                si = inst.sync_info
                waits = list(si.on_wait) if si and si.on_wait else []
                if len(waits) > 1:
                    for w in waits[:-1]:
                        nop = mybir.InstNoOp(
                            name=self.nc.get_next_instruction_name(),
                            engine=inst.engine,
                            bass_nofuse=True,
                            sync_info=bass_rust.SyncInfo(on_wait=[w], on_update=[]),
                        )
                        self.nc.register_instruction(nop)
                        expanded.append(nop)
                    inst.sync_info = bass_rust.SyncInfo(
                        on_wait=[waits[-1]],
                        on_update=list(si.on_update) if si.on_update else [],
                    )
                expanded.append(inst)
            insts[:] = expanded
        return orig_lower(self, ordered)

    tile.TileContext._lower_ordered_insts = _lower_with_wait_split
    tile.TileContext._drain_patched = True


def build_nc():
    _patch_tile_drain()
    nc = bass.Bass()
    # partition-major tiled layouts (see shard_inputs)
    enc8_t = nc.declare_dram_parameter(
        "enc8_t", [BL, NLC, 2, 128, KH, LC], F8, isOutput=False
    )
    encb_t = nc.declare_dram_parameter(
        "encb_t", [BL, 2, 128, KH, L], BF16, isOutput=False
    )
    wenc8_d = nc.declare_dram_parameter(
        "wenc8", [128, KT, DEC], F8, isOutput=False
    )
    bias_d = nc.declare_dram_parameter("bias_kpb", [128, DT, BL], F32, isOutput=False)
    v_mat = nc.declare_dram_parameter("v_mat", [128, DT, 16], F8, isOutput=False)
    eye = nc.declare_dram_parameter("eye", [128, 128], F32, isOutput=False)
    ones = nc.declare_dram_parameter("ones", [1, 128], BF16, isOutput=False)
    att = nc.declare_dram_parameter("att", [BL, ENC2], F32, isOutput=True)

    with tile.TileContext(nc) as tc, ExitStack() as ctx:
        singles = ctx.enter_context(tc.tile_pool(name="singles", bufs=1))
        smalls = ctx.enter_context(tc.tile_pool(name="smalls", bufs=2))
        enc8_pool = ctx.enter_context(tc.tile_pool(name="enc8", bufs=8))
        encb_pool = ctx.enter_context(tc.tile_pool(name="encb", bufs=6))
        energy_pool = ctx.enter_context(tc.tile_pool(name="energy", bufs=3))
        wexp_pool = ctx.enter_context(tc.tile_pool(name="wexp", bufs=2))
        prod_pool = ctx.enter_context(tc.tile_pool(name="prod", bufs=2))
        ep_ps = ctx.enter_context(tc.tile_pool(name="ep_ps", bufs=3, space="PSUM"))
        sc_ps = ctx.enter_context(tc.tile_pool(name="sc_ps", bufs=2, space="PSUM"))
        wb_ps = ctx.enter_context(tc.tile_pool(name="wb_ps", bufs=2, space="PSUM"))
        att_ps_pool = ctx.enter_context(
            tc.tile_pool(name="att_ps", bufs=1, space="PSUM")
        )

        # ---- persistent tiles -------------------------------------------
        wenc = singles.tile([128, KT, DEC], F8)  # W_enc*128, (e-tile, k) x d
        v_sb = singles.tile([128, DT, 16], F8)
        eye_sb = singles.tile([128, 128], F32)
        bias_sb = singles.tile([128, DT, BL], F32)  # dec_proj + b_score
        att_all = singles.tile([128, KT * BL], F32)  # att^T cols = b*KT+k
        ones_sb = singles.tile([1, 128], BF16)

        def alloc_chunk8(nm):
            a = enc8_pool.tile([128, KH, LC], F8, tag="enc", name=f"{nm}a")
            bb = enc8_pool.tile([128, KH, LC], F8, tag="enc", name=f"{nm}b")
            return (a, bb)

        def load_chunk8(b, c, ch):
            for h in range(2):
                nc.sync.dma_start(out=ch[h], in_=enc8_t[b, c, h])

        def alloc_encb(nm):
            a = encb_pool.tile([128, KH, L], BF16, tag="encb", name=f"{nm}a")
            bb = encb_pool.tile([128, KH, L], BF16, tag="encb", name=f"{nm}b")
            return (a, bb)

        def load_encb_parts(b, ch, parts):
            for part in parts:
                h, q = part // 2, (part % 2) * (KH // 2)
                nc.sync.dma_start(
                    out=ch[h][:, q : q + KH // 2, :],
                    in_=encb_t[b, h][:, q : q + KH // 2, :],
                )

        def enc_pair(ch, kp):
            """[128, 2, LC] DoubleRow rhs slice for pair (2kp, 2kp+1)."""
            k = 2 * kp
            t, kk = (ch[0], k) if k < KH else (ch[1], k - KH)
            return t[:, kk : kk + 2, :]

        def encb_sl(ch, k, lo=0, width=L):
            t, kk = (ch[0], k) if k < KH else (ch[1], k - KH)
            return t[:, kk, lo : lo + width]

        def w_pair(kp, dt):
            """[128, 2, 128] DoubleRow lhsT slice."""
            return wenc[:, 2 * kp : 2 * kp + 2, dt * 128 : (dt + 1) * 128]

        # ---- startup DMA, in data-arrival order -------------------------
        enc00 = alloc_chunk8("enc00")
        nc.sync.dma_start(out=enc00[0][:, 0:2, :], in_=enc8_t[0, 0, 0][:, 0:2, :])
        nc.sync.dma_start(out=wenc[:, 0:2, :], in_=wenc8_d[:, 0:2, :])
        nc.sync.dma_start(out=enc00[0][:, 2:4, :], in_=enc8_t[0, 0, 0][:, 2:4, :])
        nc.sync.dma_start(out=wenc[:, 2:4, :], in_=wenc8_d[:, 2:4, :])
        nc.sync.dma_start(out=enc00[0][:, 4:8, :], in_=enc8_t[0, 0, 0][:, 4:8, :])
        nc.sync.dma_start(out=wenc[:, 4:8, :], in_=wenc8_d[:, 4:8, :])
        nc.sync.dma_start(out=bias_sb, in_=bias_d[:, :, :])
        nc.sync.dma_start(out=v_sb, in_=v_mat[:, :, :])
        nc.sync.dma_start(out=enc00[1], in_=enc8_t[0, 0, 1])
        nc.sync.dma_start(out=wenc[:, 8:12, :], in_=wenc8_d[:, 8:12, :])
        nc.sync.dma_start(out=wenc[:, 12:16, :], in_=wenc8_d[:, 12:16, :])
        nc.sync.dma_start(out=eye_sb, in_=eye[:, :])
        nc.sync.dma_start(out=ones_sb, in_=ones[:, :])

        # ---- chunk (0,0): consume pairs in half-arrival order ----------
        ps00 = {
            dt: ep_ps.tile([128, LC], F32, tag="ep", name=f"ps00_{dt}")
            for dt in range(3)
        }
        for kp in range(KP):
            for dt in range(3):
                nc.tensor.matmul(
                    ps00[dt],
                    lhsT=w_pair(kp, dt),
                    rhs=enc_pair(enc00, kp),
                    start=(kp == 0),
                    stop=(kp == KP - 1),
                    perf_mode=PM.DoubleRow,
                )

        def tanh_pair(ps, energy, b, dt):
            """tanh one d-tile's psum into half of an fp8 pair tile."""
            nc.scalar.activation(
                out=energy[:, dt % 2, :],
                in_=ps,
                func=AF.Tanh,
                bias=bias_sb[:, dt, b : b + 1],
                scale=1.0 / WSCALE,
            )

        def score_pair(sc, energy, dtp):
            """DoubleRow matvec over one d-tile pair: sc += v . energy."""
            nc.tensor.matmul(
                sc,
                lhsT=v_sb[:, 2 * dtp : 2 * dtp + 2, 0:1],
                rhs=energy,
                start=(dtp == 0),
                stop=(dtp == DT // 2 - 1),
                perf_mode=PM.DoubleRow,
            )

        def alloc_energy(nm):
            return energy_pool.tile([128, 2, LC], F8, tag="energy", name=nm)

        sc00 = sc_ps.tile([1, LC], F32, tag="sc")
        en00 = {}
        for dtp in range(DT // 2):
            en00[dtp] = None
        cur_en = alloc_energy("en00_0")
        tanh_pair(ps00[0], cur_en, 0, 0)
        tanh_pair(ps00[1], cur_en, 0, 1)
        score_pair(sc00, cur_en, 0)
        cur_en = alloc_energy("en00_1")
        tanh_pair(ps00[2], cur_en, 0, 2)
        for dt in range(3, DT):
            ps = ep_ps.tile([128, LC], F32, tag="ep", name=f"ps00b_{dt}")
            for kp in range(KP):
                nc.tensor.matmul(
                    ps,
                    lhsT=w_pair(kp, dt),
                    rhs=enc_pair(enc00, kp),
                    start=(kp == 0),
                    stop=(kp == KP - 1),
                    perf_mode=PM.DoubleRow,
                )
            if dt % 2 == 0:
                cur_en = alloc_energy(f"en00_{dt // 2}")
            tanh_pair(ps, cur_en, 0, dt)
            if dt % 2 == 1:
                score_pair(sc00, cur_en, dt // 2)

        def make_chunk(b, c, enc_tile):
            """Return (sc, group emitters, matvec emitters) for one
            chunk. The caller interleaves them with the previous chunk's
            tail ops (software pipelining) so ACT latencies never sit
            exposed in the in-order PE stream."""
            sc = sc_ps.tile([1, LC], F32, tag="sc", name=f"sc_{b}_{c}")
            ens = {}

            def make_group(dtp):
                def emit():
                    en = alloc_energy(f"en_{b}_{c}_{dtp}")
                    for i in range(2):
                        dt = 2 * dtp + i
                        ps = ep_ps.tile(
                            [128, LC], F32, tag="ep", name=f"dm_{b}_{c}_{dt}"
                        )
                        for kp in range(KP):
                            nc.tensor.matmul(
                                ps,
                                lhsT=w_pair(kp, dt),
                                rhs=enc_pair(enc_tile, kp),
                                start=(kp == 0),
                                stop=(kp == KP - 1),
                                perf_mode=PM.DoubleRow,
                            )
                        tanh_pair(ps, en, b, dt)
                    ens[dtp] = en

                return emit

            def make_mv(dtp):
                def emit():
                    score_pair(sc, ens[dtp], dtp)

                return emit

            groups = [make_group(p) for p in range(DT // 2)]
            mvs = [make_mv(p) for p in range(DT // 2)]
            return sc, groups, mvs

        def score_to_wexp(sc, wexp, b, c):
            """Exp the raw chunk scores into the batch's broadcast weight
            tile; returns the per-partition denominator contribution."""
            s_sb = smalls.tile([1, LC], BF16, tag="ssb", name=f"ssb_{b}_{c}")
            nc.scalar.copy(out=s_sb, in_=sc)
            wb = wb_ps.tile([128, LC], F32, tag="wb", name=f"wb_{b}_{c}")
            nc.tensor.matmul(wb, lhsT=ones_sb, rhs=s_sb, start=True, stop=True)
            den_c = smalls.tile([128, 1], F32, tag=f"den{c}", name=f"den_{b}_{c}")
            nc.scalar.activation(
                out=wexp[:, c * LC : (c + 1) * LC],
                in_=wb,
                func=AF.Exp,
                bias=0.0,
                scale=1.0 / VSCALE,
                accum_out=den_c,
            )
            return den_c

        def batch_att(b, encb_tile, wexp):
            """Fused weighted reduce over the full L per k-tile, split
            ~3:1 between DVE (fused STT) and ACT (accum-copy over DVE
            pair-products). Emits the DVE work now; returns a closure
            with the ACT half so the caller can defer it past the next
            chunk's tanh chain (else ACT bunches and PE starves on psum
            banks)."""
            w_pairbc = bass.AP(
                tensor=wexp.tensor,
                offset=wexp.offset,
                ap=[wexp.ap[0], [0, 2], wexp.ap[1]],
            )
            for k in range(KT - 2):
                col = b * KT + k
                prod = prod_pool.tile([128, L], BF16, tag="prod", name=f"pr_{b}_{k}")
                nc.vector.scalar_tensor_tensor(
                    out=prod,
                    in0=encb_sl(encb_tile, k),
                    scalar=1.0,
                    in1=wexp,
                    op0=ALU.mult,
                    op1=ALU.mult,
                    accum_out=att_all[:, col : col + 1],
                )
            pprods = []
            for kp in range(KT // 2 - 1, KT // 2):  # k 14..15 -> ACT
                k = 2 * kp
                t, kk = (encb_tile[0], k) if k < KH else (encb_tile[1], k - KH)
                prod = prod_pool.tile(
                    [128, 2, L], BF16, tag="prodp", name=f"prp_{b}_{kp}"
                )
                nc.vector.tensor_mul(out=prod, in0=t[:, kk : kk + 2, :], in1=w_pairbc)
                pprods.append((k, prod))

            def act_half():
                scr = smalls.tile([128, L], BF16, tag="ascr", name=f"ascr_b{b}")
                for k, prod in pprods:
                    for i in range(2):
                        col = b * KT + k + i
                        nc.scalar.activation(
                            out=scr,
                            in_=prod[:, i, :],
                            func=AF.Copy,
                            bias=0.0,
                            scale=1.0,
                            accum_out=att_all[:, col : col + 1],
                        )

            return act_half

        def chunk_att_tail(b, c, encb_tile, wexp, n_act=4):
            """Kernel-tail variant: split the reduce between DVE (fused
            STT) and ACT (accum-copy over DVE pair-products, last n_act
            k-tiles) so the exposed tail shortens. Emitted inline: the
            ACT half overlaps the next chunk's window via queue slack."""
            atmp = None
            if c > 0:
                atmp = smalls.tile([128, KT], F32, tag="atmp", name=f"atmpt_{b}_{c}")

            def dst(k):
                if c == 0:
                    col = b * KT + k
                    return att_all[:, col : col + 1]
                return atmp[:, k : k + 1]

            w_sl = wexp[:, c * LC : (c + 1) * LC]
            w_pairbc = bass.AP(
                tensor=w_sl.tensor,
                offset=w_sl.offset,
                ap=[w_sl.ap[0], [0, 2], w_sl.ap[1]],
            )
            pprods = []
            for kp in range(KT // 2 - n_act // 2, KT // 2):
                k = 2 * kp
                t, kk = (encb_tile[0], k) if k < KH else (encb_tile[1], k - KH)
                prod = prod_pool.tile(
                    [128, 2, LC], BF16, tag="prodt", name=f"prp_{b}_{c}_{kp}"
                )
                nc.vector.tensor_mul(
                    out=prod,
                    in0=t[:, kk : kk + 2, c * LC : (c + 1) * LC],
                    in1=w_pairbc,
                )
                pprods.append((k, prod))
            for k in range(KT - n_act):
                prod = prod_pool.tile(
                    [128, L], BF16, tag="prod", name=f"prt_{b}_{c}_{k}"
                )
                nc.vector.scalar_tensor_tensor(
                    out=prod[:, 0:LC],
                    in0=encb_sl(encb_tile, k, c * LC, LC),
                    scalar=1.0,
                    in1=w_sl,
                    op0=ALU.mult,
                    op1=ALU.mult,
                    accum_out=dst(k),
                )

            scr = smalls.tile([128, LC], BF16, tag="ascr", name=f"ascr_{b}_{c}")
            for k, prod in pprods:
                for i in range(2):
                    nc.scalar.activation(
                        out=scr,
                        in_=prod[:, i, :],
                        func=AF.Copy,
                        bias=0.0,
                        scale=1.0,
                        accum_out=dst(k + i),
                    )
            if c > 0:
                cols = slice(b * KT, (b + 1) * KT)
                nc.vector.tensor_add(
                    out=att_all[:, cols], in0=att_all[:, cols], in1=atmp
                )

        def batch_epilogue(b, dens):
            """Transpose the raw attention columns and store, folding the
            softmax normalization into the ACT psum->sbuf copy (scale)."""
            rden = smalls.tile([128, 1], F32, tag="rden")
            nc.vector.tensor_add(out=rden, in0=dens[0], in1=dens[1])
            for extra in dens[2:]:
                nc.vector.tensor_add(out=rden, in0=rden, in1=extra)
            nc.vector.reciprocal(out=rden, in_=rden)
            cols = slice(b * KT, (b + 1) * KT)
            att_bt = att_ps_pool.tile([KT, 128], F32, tag="abt")
            nc.tensor.transpose(att_bt, att_all[:, cols], eye_sb)
            att_sb = smalls.tile([KT, 128], F32, tag="asb")
            nc.scalar.activation(
                out=att_sb,
                in_=att_bt,
                func=AF.Copy,
                bias=0.0,
                scale=rden[0:KT, :],
            )
            nc.sync.dma_start(
                out=att[b].rearrange("(k p) -> k p", p=128), in_=att_sb
            )

        # ---- main loop: 1-chunk software pipeline -----------------------
        # Emission per iteration: [g0, prev.mv3, g1, prev bookkeeping
        # (scores->exp, reduces, epilogues), mv0, g2, mv1, g3, mv2] so
        # the previous chunk's ACT-latency tail hides behind this
        # chunk's matmul groups in the in-order PE stream.
        wexp_map = {}
        encb_map = {}
        state = {"dens": [], "prev_dens": None, "pending": []}

        def bookkeeping(pb, pc, sc):
            if pc == 0:
                wexp_map[pb] = wexp_pool.tile(
                    [128, L], BF16, tag="wexp", name=f"wexp_{pb}"
                )
                state["dens"] = []
            wexp = wexp_map[pb]
            state["dens"].append(score_to_wexp(sc, wexp, pb, pc))
            for fn in state["pending"]:
                fn()
            state["pending"] = []
            if pc == 1 and pb > 0:
                batch_epilogue(pb - 1, state["prev_dens"])
            if pb >= BL - 2:
                n_act = 6 if (pb == BL - 1 and pc == 1) else 4
                chunk_att_tail(pb, pc, encb_map[pb], wexp, n_act=n_act)
            elif pc == 1:
                state["pending"].append(batch_att(pb, encb_map[pb], wexp))
            if pc == 1:
                state["prev_dens"] = state["dens"]

        prev_bk = (0, 0, sc00)
        prev_mv3 = None
        for b in range(BL):
            for c in range(NLC):
                if (b, c) == (0, 0):
                    continue
                enc_tile = alloc_chunk8(f"enc_{b}_{c}")
                load_chunk8(b, c, enc_tile)
                # bf16-copy prefetch: ~2MB per chunk slot, pulled in
                # for the last two batches (their per-chunk tails read
                # all k-tiles one chunk earlier than batch-wide reduces)
                def _encb(bb, parts, alloc=False):
                    if alloc:
                        encb_map[bb] = alloc_encb(f"encb_{bb}")
                    load_encb_parts(bb, encb_map[bb], parts)

                sched = {
                    (0, 1): [(0, [0, 1], True)],
                    (1, 0): [(0, [2, 3]), (1, [0], True)],
                    (1, 1): [(1, [1, 2])],
                    (2, 0): [(1, [3]), (2, [0], True)],
                    (2, 1): [(2, [1, 2])],
                    (3, 0): [(2, [3]), (3, [0], True)],
                    (3, 1): [(3, [1, 2])],
                    (4, 0): [(3, [3]), (4, [0], True)],
                    (4, 1): [(4, [1, 2])],
                    (5, 0): [(4, [3]), (5, [0, 1], True)],
                    (5, 1): [(5, [2, 3])],
                    (6, 0): [(6, [0, 1], True)],
                    (6, 1): [(6, [2, 3]), (7, [0], True)],
                    (7, 0): [(7, [1, 2, 3])],
                }
                for item in sched.get((b, c), []):
                    _encb(item[0], item[1], len(item) > 2 and item[2])
                sc, groups, mvs = make_chunk(b, c, enc_tile)
                groups[0]()
                if prev_mv3 is not None:
                    prev_mv3()
                groups[1]()
                bookkeeping(*prev_bk)
                mvs[0]()
                groups[2]()
                mvs[1]()
                groups[3]()
                mvs[2]()
                prev_bk = (b, c, sc)
                prev_mv3 = mvs[3]
        prev_mv3()
        bookkeeping(*prev_bk)
        for fn in state["pending"]:
            fn()
        batch_epilogue(BL - 1, state["prev_dens"])

    return nc


def shard_inputs(dec_hidden, enc_output, W_score, b_score, v):
    """Full inputs -> per-core input maps (host-side layout staging)."""
    dec_hidden = np.ascontiguousarray(dec_hidden, dtype=np.float32)
    W_score = np.asarray(W_score, dtype=np.float32)
    # dec_proj + b_score computed host-side (0.05% of the FLOPs)
    bias_full = dec_hidden @ W_score[:DEC] + np.asarray(b_score, dtype=np.float32)
    # W_enc tiled partition-major: [p, k, d]
    wenc8 = np.ascontiguousarray(
        (W_score[DEC:] * WSCALE).reshape(KT, 128, DEC).transpose(1, 0, 2)
    ).astype(ml_dtypes.float8_e4m3)
    v_pd = np.asarray(v, dtype=np.float32).reshape(DT, 128).T * VSCALE
    v_mat = np.zeros((128, DT, 16), dtype=ml_dtypes.float8_e4m3)
    v_mat[:, :, 0] = v_pd.astype(ml_dtypes.float8_e4m3)
    eye = np.eye(128, dtype=np.float32)

    in_maps = []
    for core in range(N_CORES):
        sl = slice(core * BL, (core + 1) * BL)
        # (L, BL, 2E) -> (BL, 2E, L)
        enc_t = np.ascontiguousarray(
            np.asarray(enc_output[:, sl, :], dtype=np.float32).transpose(1, 2, 0)
        )
        # fp8 chunk-major partition-tiled: [b, c, half, p, k, l]
        enc8_t = np.ascontiguousarray(
            enc_t.reshape(BL, 2, KH, 128, NLC, LC).transpose(0, 4, 1, 3, 2, 5)
        ).astype(ml_dtypes.float8_e4m3)
        # bf16 full-L partition-tiled: [b, half, p, k, l]
        encb_t = np.ascontiguousarray(
            enc_t.reshape(BL, 2, KH, 128, L).transpose(0, 1, 3, 2, 4)
        ).astype(ml_dtypes.bfloat16)
        # (BL, DEC) -> [p, dt, b]
        bias_kpb = np.ascontiguousarray(
            bias_full[sl].T.reshape(DT, 128, BL).transpose(1, 0, 2)
        )
        in_maps.append(
            {
                "enc8_t": enc8_t,
                "encb_t": encb_t,
                "ones": np.ones((1, 128), dtype=ml_dtypes.bfloat16),
                "bias_kpb": bias_kpb,
                "wenc8": wenc8,
                "v_mat": v_mat,
                "eye": eye,
            }
        )
    return in_maps


_NC_CACHE = None


def kernel(dec_hidden, enc_output, W_score, b_score, v):
    global _NC_CACHE
    if _NC_CACHE is None:
        _NC_CACHE = build_nc()
    nc = _NC_CACHE
    in_maps = shard_inputs(dec_hidden, enc_output, W_score, b_score, v)
    res = run_bass_kernel_spmd(nc, in_maps, list(range(N_CORES)))
    return np.concatenate([res.results[i]["att"] for i in range(N_CORES)], axis=0)


# revision 47
# speedup vs baseline: 1.2182x; 1.0220x over previous
"""Bahdanau additive-attention kernel for Trainium2, data-parallel over
batch across 8 NeuronCores.

Per batch b:
    energy  = tanh(dec_proj[b] + enc[b] @ W_enc + b_score)   # (L, DEC)
    scores  = energy @ v                                     # (L,)
    alpha   = softmax(scores)
    att[b]  = alpha @ enc[b]                                 # (2E,)

On-device layout (per core, 8 batches):
  - enc is staged host-side TWICE, both partition-major-tiled so DMA
    descriptors are 4-16KB runs: enc8_t (fp8e4m3, chunk-major) feeds the
    PE energy matmul in perf_mode=DoubleRow (two k-subtiles per
    instruction, measured ~2x over bf16/f32r at N=512); encb_t (bf16,
    full-L tiles) feeds the DVE attention reduce. W_enc is pre-scaled by
    128 on the host so its values sit mid-range in e4m3; the tanh
    activation rescales the psum by 1/128.
  - dec_proj + b_score fold into a host-computed bias (0.05% of FLOPs);
    the tanh activation applies it per-partition.
  - energy is written fp8 in d-tile pairs; scores accumulate via
    DoubleRow matvecs against v*32 (16B-padded pair layout).
  - softmax skips the max-subtraction: |scores| <= 32*VSCALE, safely
    inside the fp32 exp range. Raw scores broadcast to 128 partitions
    with a K=1 bf16 ones-matmul; Exp (scale 1/VSCALE) writes the bf16
    weight tile with accum_out giving the replicated denominator.
  - att^T accumulates via fused scalar_tensor_tensor on DVE:
    accum_out[e,1] = sum_l enc_bf16[e,l] * exp_scores[l], one full-L
    instruction per (batch, k-tile); ~1/8 of k-tiles ride ACT as
    accum-copies over DVE pair-products. The last two batches run
    per-chunk with a larger ACT share so only half of one chunk's
    reduce sits in the kernel tail.
  - the main loop is a 1-chunk software pipeline: each chunk's last
    matvec, score-exp path, reduces, and the previous batch's
    transpose epilogue emit between the next chunk's matmul groups, so
    ACT/DVE latencies never sit exposed in the in-order PE stream.
"""

import numpy as np
import ml_dtypes
from contextlib import ExitStack

import concourse.bass as bass
import concourse.tile as tile
from concourse import mybir
from concourse.bass_utils import run_bass_kernel_spmd
from concourse.vector_clock import ScopedClock, VectorClock

N_CORES = 8
B, L, DEC, ENC2 = 64, 1024, 1024, 2048
BL = B // N_CORES  # batches per core
KT = ENC2 // 128   # contraction tiles over e
KP = KT // 2       # DoubleRow pairs over e
KH = KT // 2       # k-tiles per half (SBUF tile granularity)
DT = DEC // 128    # d tiles
LC = 512           # l-chunk (one PSUM bank of f32)
NLC = L // LC
WSCALE = 128.0     # host-side W_enc scaling for fp8 range
VSCALE = 32.0      # host-side v scaling for fp8 range

F32 = mybir.dt.float32
F32R = mybir.dt.float32r
F8 = mybir.dt.float8e4
BF16 = mybir.dt.bfloat16
AF = mybir.ActivationFunctionType
ALU = mybir.AluOpType
PM = mybir.MatmulPerfMode


def _patch_tile_drain():
    """Workarounds for this container's walrus build.

    1. The Tile tail drain carries one sem wait per touched proc; walrus
       rejects >2 on the CTRL encoding. Split the waits onto single-wait
       SP nops (SP executes in order, so the drain then needs none).
    2. Any instruction with 2+ sem waits can fail codegen (the matmul
       LW encoding holds a single wait). Split multi-wait instructions:
       excess waits move onto same-engine InstNoOp carriers inserted
       just before; engine program order makes this equivalent.
    """
    if getattr(tile.TileContext, "_drain_patched", False):
        return

    def _drain_and_barrier(self, tick_clock, wait_clock):
        vec = list(tick_clock.global_clock)
        n = len(vec)
        for i in range(n):
            if vec[i] <= 0:
                continue
            part = [0] * n
            part[i] = vec[i]
            nop_inst = self.nc.sync.nop(nofuse=True)
            wait_clock.add_sem_waits(
                nop_inst.ins, ScopedClock({None: VectorClock(part)})
            )
        self.nc.sync.drain()
        self.nc.all_engine_barrier()
        assert self.sems is not None
        popped = self.nc._tile_sem_poison_stack.pop()
        assert popped is self._sem_poison
        self.nc.clear_and_free_semaphores(list(self.sems.allocated().values()))
        self.nc.all_engine_barrier()

    tile.TileContext._drain_and_barrier = _drain_and_barrier

    import bass_rust

    orig_lower = tile.TileContext._lower_ordered_insts

    def _lower_with_wait_split(self, ordered):
        for insts in ordered.values():
            expanded = []
            for inst in insts:
                si = inst.sync_info
                waits = list(si.on_wait) if si and si.on_wait else []
                if len(waits) > 1:
                    for w in waits[:-1]:
                        nop = mybir.InstNoOp(
                            name=self.nc.get_next_instruction_name(),
                            engine=inst.engine,
                            bass_nofuse=True,
                            sync_info=bass_rust.SyncInfo(on_wait=[w], on_update=[]),
                        )
                        self.nc.register_instruction(nop)
                        expanded.append(nop)
                    inst.sync_info = bass_rust.SyncInfo(
                        on_wait=[waits[-1]],
                        on_update=list(si.on_update) if si.on_update else [],
                    )
                expanded.append(inst)
            insts[:] = expanded
        return orig_lower(self, ordered)

    tile.TileContext._lower_ordered_insts = _lower_with_wait_split
    tile.TileContext._drain_patched = True


def build_nc():
    _patch_tile_drain()
    nc = bass.Bass()
    # partition-major tiled layouts (see shard_inputs)
    enc8_t = nc.declare_dram_parameter(
        "enc8_t", [BL, NLC, 2, 128, KH, LC], F8, isOutput=False
    )
    encb_t = nc.declare_dram_parameter(
        "encb_t", [BL, 2, 128, KH, L], BF16, isOutput=False
    )
    wenc8_d = nc.declare_dram_parameter(
        "wenc8", [128, KT, DEC], F8, isOutput=False
    )
    bias_d = nc.declare_dram_parameter("bias_kpb", [128, DT, BL], F32, isOutput=False)
    v_mat = nc.declare_dram_parameter("v_mat", [128, DT, 16], F8, isOutput=False)
    eye = nc.declare_dram_parameter("eye", [128, 128], F32, isOutput=False)
    ones = nc.declare_dram_parameter("ones", [1, 128], BF16, isOutput=False)
    att = nc.declare_dram_parameter("att", [BL, ENC2], F32, isOutput=True)

    with tile.TileContext(nc) as tc, ExitStack() as ctx:
        singles = ctx.enter_context(tc.tile_pool(name="singles", bufs=1))
        smalls = ctx.enter_context(tc.tile_pool(name="smalls", bufs=2))
        enc8_pool = ctx.enter_context(tc.tile_pool(name="enc8", bufs=8))
        encb_pool = ctx.enter_context(tc.tile_pool(name="encb", bufs=6))
        energy_pool = ctx.enter_context(tc.tile_pool(name="energy", bufs=3))
        wexp_pool = ctx.enter_context(tc.tile_pool(name="wexp", bufs=2))
        prod_pool = ctx.enter_context(tc.tile_pool(name="prod", bufs=2))
        ep_ps = ctx.enter_context(tc.tile_pool(name="ep_ps", bufs=3, space="PSUM"))
        sc_ps = ctx.enter_context(tc.tile_pool(name="sc_ps", bufs=2, space="PSUM"))
        wb_ps = ctx.enter_context(tc.tile_pool(name="wb_ps", bufs=2, space="PSUM"))
        att_ps_pool = ctx.enter_context(
            tc.tile_pool(name="att_ps", bufs=1, space="PSUM")
        )

        # ---- persistent tiles -------------------------------------------
        wenc = singles.tile([128, KT, DEC], F8)  # W_enc*128, (e-tile, k) x d
        v_sb = singles.tile([128, DT, 16], F8)
        eye_sb = singles.tile([128, 128], F32)
        bias_sb = singles.tile([128, DT, BL], F32)  # dec_proj + b_score
        att_all = singles.tile([128, KT * BL], F32)  # att^T cols = b*KT+k
        ones_sb = singles.tile([1, 128], BF16)

        def alloc_chunk8(nm):
            a = enc8_pool.tile([128, KH, LC], F8, tag="enc", name=f"{nm}a")
            bb = enc8_pool.tile([128, KH, LC], F8, tag="enc", name=f"{nm}b")
            return (a, bb)

        def load_chunk8(b, c, ch):
            for h in range(2):
                nc.sync.dma_start(out=ch[h], in_=enc8_t[b, c, h])

        def alloc_encb(nm):
            a = encb_pool.tile([128, KH, L], BF16, tag="encb", name=f"{nm}a")
            bb = encb_pool.tile([128, KH, L], BF16, tag="encb", name=f"{nm}b")
            return (a, bb)

        def load_encb_parts(b, ch, parts):
            for part in parts:
                h, q = part // 2, (part % 2) * (KH // 2)
                nc.sync.dma_start(
                    out=ch[h][:, q : q + KH // 2, :],
                    in_=encb_t[b, h][:, q : q + KH // 2, :],
                )

        def enc_pair(ch, kp):
            """[128, 2, LC] DoubleRow rhs slice for pair (2kp, 2kp+1)."""
            k = 2 * kp
            t, kk = (ch[0], k) if k < KH else (ch[1], k - KH)
            return t[:, kk : kk + 2, :]

        def encb_sl(ch, k, lo=0, width=L):
            t, kk = (ch[0], k) if k < KH else (ch[1], k - KH)
            return t[:, kk, lo : lo + width]

        def w_pair(kp, dt):
            """[128, 2, 128] DoubleRow lhsT slice."""
            return wenc[:, 2 * kp : 2 * kp + 2, dt * 128 : (dt + 1) * 128]

        # ---- startup DMA, in data-arrival order -------------------------
        enc00 = alloc_chunk8("enc00")
        nc.sync.dma_start(out=enc00[0][:, 0:2, :], in_=enc8_t[0, 0, 0][:, 0:2, :])
        nc.sync.dma_start(out=wenc[:, 0:2, :], in_=wenc8_d[:, 0:2, :])
        nc.sync.dma_start(out=enc00[0][:, 2:4, :], in_=enc8_t[0, 0, 0][:, 2:4, :])
        nc.sync.dma_start(out=wenc[:, 2:4, :], in_=wenc8_d[:, 2:4, :])
        nc.sync.dma_start(out=enc00[0][:, 4:8, :], in_=enc8_t[0, 0, 0][:, 4:8, :])
        nc.sync.dma_start(out=wenc[:, 4:8, :], in_=wenc8_d[:, 4:8, :])
        nc.sync.dma_start(out=bias_sb, in_=bias_d[:, :, :])
        nc.sync.dma_start(out=v_sb, in_=v_mat[:, :, :])
        nc.sync.dma_start(out=enc00[1], in_=enc8_t[0, 0, 1])
        nc.sync.dma_start(out=wenc[:, 8:12, :], in_=wenc8_d[:, 8:12, :])
        nc.sync.dma_start(out=wenc[:, 12:16, :], in_=wenc8_d[:, 12:16, :])
        nc.sync.dma_start(out=eye_sb, in_=eye[:, :])
        nc.sync.dma_start(out=ones_sb, in_=ones[:, :])

        # ---- chunk (0,0): consume pairs in half-arrival order ----------
        ps00 = {
            dt: ep_ps.tile([128, LC], F32, tag="ep", name=f"ps00_{dt}")
            for dt in range(3)
        }
        for kp in range(KP):
            for dt in range(3):
                nc.tensor.matmul(
                    ps00[dt],
                    lhsT=w_pair(kp, dt),
                    rhs=enc_pair(enc00, kp),
                    start=(kp == 0),
                    stop=(kp == KP - 1),
                    perf_mode=PM.DoubleRow,
                )

        def tanh_pair(ps, energy, b, dt):
            """tanh one d-tile's psum into half of an fp8 pair tile."""
            nc.scalar.activation(
                out=energy[:, dt % 2, :],
                in_=ps,
                func=AF.Tanh,
                bias=bias_sb[:, dt, b : b + 1],
                scale=1.0 / WSCALE,
            )

        def score_pair(sc, energy, dtp):
            """DoubleRow matvec over one d-tile pair: sc += v . energy."""
            nc.tensor.matmul(
                sc,
                lhsT=v_sb[:, 2 * dtp : 2 * dtp + 2, 0:1],
                rhs=energy,
                start=(dtp == 0),
                stop=(dtp == DT // 2 - 1),
                perf_mode=PM.DoubleRow,
            )

        def alloc_energy(nm):
            return energy_pool.tile([128, 2, LC], F8, tag="energy", name=nm)

        sc00 = sc_ps.tile([1, LC], F32, tag="sc")
        en00 = {}
        for dtp in range(DT // 2):
            en00[dtp] = None
        cur_en = alloc_energy("en00_0")
        tanh_pair(ps00[0], cur_en, 0, 0)
        tanh_pair(ps00[1], cur_en, 0, 1)
        score_pair(sc00, cur_en, 0)
        cur_en = alloc_energy("en00_1")
        tanh_pair(ps00[2], cur_en, 0, 2)
        for dt in range(3, DT):
            ps = ep_ps.tile([128, LC], F32, tag="ep", name=f"ps00b_{dt}")
            for kp in range(KP):
                nc.tensor.matmul(
                    ps,
                    lhsT=w_pair(kp, dt),
                    rhs=enc_pair(enc00, kp),
                    start=(kp == 0),
                    stop=(kp == KP - 1),
                    perf_mode=PM.DoubleRow,
                )
            if dt % 2 == 0:
                cur_en = alloc_energy(f"en00_{dt // 2}")
            tanh_pair(ps, cur_en, 0, dt)
            if dt % 2 == 1:
                score_pair(sc00, cur_en, dt // 2)

        def make_chunk(b, c, enc_tile):
            """Return (sc, group emitters, matvec emitters) for one
            chunk. The caller interleaves them with the previous chunk's
            tail ops (software pipelining) so ACT latencies never sit
            exposed in the in-order PE stream."""
            sc = sc_ps.tile([1, LC], F32, tag="sc", name=f"sc_{b}_{c}")
            ens = {}

            def make_group(dtp):
                def emit():
                    en = alloc_energy(f"en_{b}_{c}_{dtp}")
                    for i in range(2):
                        dt = 2 * dtp + i
                        ps = ep_ps.tile(
                            [128, LC], F32, tag="ep", name=f"dm_{b}_{c}_{dt}"
                        )
                        for kp in range(KP):
                            nc.tensor.matmul(
                                ps,
                                lhsT=w_pair(kp, dt),
                                rhs=enc_pair(enc_tile, kp),
                                start=(kp == 0),
                                stop=(kp == KP - 1),
                                perf_mode=PM.DoubleRow,
                            )
                        tanh_pair(ps, en, b, dt)
                    ens[dtp] = en

                return emit

            def make_mv(dtp):
                def emit():
                    score_pair(sc, ens[dtp], dtp)

                return emit

            groups = [make_group(p) for p in range(DT // 2)]
            mvs = [make_mv(p) for p in range(DT // 2)]
            return sc, groups, mvs

        def score_to_wexp(sc, wexp, b, c):
            """Exp the raw chunk scores into the batch's broadcast weight
            tile; returns the per-partition denominator contribution."""
            s_sb = smalls.tile([1, LC], BF16, tag="ssb", name=f"ssb_{b}_{c}")
            nc.scalar.copy(out=s_sb, in_=sc)
            wb = wb_ps.tile([128, LC], F32, tag="wb", name=f"wb_{b}_{c}")
            nc.tensor.matmul(wb, lhsT=ones_sb, rhs=s_sb, start=True, stop=True)
            den_c = smalls.tile([128, 1], F32, tag=f"den{c}", name=f"den_{b}_{c}")
            nc.scalar.activation(
                out=wexp[:, c * LC : (c + 1) * LC],
                in_=wb,
                func=AF.Exp,
                bias=0.0,
                scale=1.0 / VSCALE,
                accum_out=den_c,
            )
            return den_c

        def batch_att(b, encb_tile, wexp):
            """Fused weighted reduce over the full L per k-tile, split
            ~3:1 between DVE (fused STT) and ACT (accum-copy over DVE
            pair-products). Emits the DVE work now; returns a closure
            with the ACT half so the caller can defer it past the next
            chunk's tanh chain (else ACT bunches and PE starves on psum
            banks)."""
            w_pairbc = bass.AP(
                tensor=wexp.tensor,
                offset=wexp.offset,
                ap=[wexp.ap[0], [0, 2], wexp.ap[1]],
            )
            for k in range(KT - 2):
                col = b * KT + k
                prod = prod_pool.tile([128, L], BF16, tag="prod", name=f"pr_{b}_{k}")
                nc.vector.scalar_tensor_tensor(
                    out=prod,
                    in0=encb_sl(encb_tile, k),
                    scalar=1.0,
                    in1=wexp,
                    op0=ALU.mult,
                    op1=ALU.mult,
                    accum_out=att_all[:, col : col + 1],
                )
            pprods = []
            for kp in range(KT // 2 - 1, KT // 2):  # k 14..15 -> ACT
                k = 2 * kp
                t, kk = (encb_tile[0], k) if k < KH else (encb_tile[1], k - KH)
                prod = prod_pool.tile(
                    [128, 2, L], BF16, tag="prodp", name=f"prp_{b}_{kp}"
                )
                nc.vector.tensor_mul(out=prod, in0=t[:, kk : kk + 2, :], in1=w_pairbc)
                pprods.append((k, prod))

            def act_half():
                scr = smalls.tile([128, L], BF16, tag="ascr", name=f"ascr_b{b}")
                for k, prod in pprods:
                    for i in range(2):
                        col = b * KT + k + i
                        nc.scalar.activation(
                            out=scr,
                            in_=prod[:, i, :],
                            func=AF.Copy,
                            bias=0.0,
                            scale=1.0,
                            accum_out=att_all[:, col : col + 1],
                        )

            return act_half

        def chunk_att_tail(b, c, encb_tile, wexp, n_act=4):
            """Kernel-tail variant: split the reduce between DVE (fused
            STT) and ACT (accum-copy over DVE pair-products, last n_act
            k-tiles) so the exposed tail shortens. Emitted inline: the
            ACT half overlaps the next chunk's window via queue slack."""
            atmp = None
            if c > 0:
                atmp = smalls.tile([128, KT], F32, tag="atmp", name=f"atmpt_{b}_{c}")

            def dst(k):
                if c == 0:
                    col = b * KT + k
                    return att_all[:, col : col + 1]
                return atmp[:, k : k + 1]

            w_sl = wexp[:, c * LC : (c + 1) * LC]
            w_pairbc = bass.AP(
                tensor=w_sl.tensor,
                offset=w_sl.offset,
                ap=[w_sl.ap[0], [0, 2], w_sl.ap[1]],
            )
            pprods = []
            for kp in range(KT // 2 - n_act // 2, KT // 2):
                k = 2 * kp
                t, kk = (encb_tile[0], k) if k < KH else (encb_tile[1], k - KH)
                prod = prod_pool.tile(
                    [128, 2, LC], BF16, tag="prodt", name=f"prp_{b}_{c}_{kp}"
                )
                nc.vector.tensor_mul(
                    out=prod,
                    in0=t[:, kk : kk + 2, c * LC : (c + 1) * LC],
                    in1=w_pairbc,
                )
                pprods.append((k, prod))
            for k in range(KT - n_act):
                prod = prod_pool.tile(
                    [128, L], BF16, tag="prod", name=f"prt_{b}_{c}_{k}"
                )
                nc.vector.scalar_tensor_tensor(
                    out=prod[:, 0:LC],
                    in0=encb_sl(encb_tile, k, c * LC, LC),
                    scalar=1.0,
                    in1=w_sl,
                    op0=ALU.mult,
                    op1=ALU.mult,
                    accum_out=dst(k),
                )

            scr = smalls.tile([128, LC], BF16, tag="ascr", name=f"ascr_{b}_{c}")
            for k, prod in pprods:
                for i in range(2):
                    nc.scalar.activation(
                        out=scr,
                        in_=prod[:, i, :],
                        func=AF.Copy,
                        bias=0.0,
                        scale=1.0,
                        accum_out=dst(k + i),
                    )
            if c > 0:
                cols = slice(b * KT, (b + 1) * KT)
                nc.vector.tensor_add(
                    out=att_all[:, cols], in0=att_all[:, cols], in1=atmp
                )

        def batch_epilogue(b, dens):
            """Transpose the raw attention columns and store, folding the
            softmax normalization into the ACT psum->sbuf copy (scale)."""
            rden = smalls.tile([128, 1], F32, tag="rden")
            nc.vector.tensor_add(out=rden, in0=dens[0], in1=dens[1])
            for extra in dens[2:]:
                nc.vector.tensor_add(out=rden, in0=rden, in1=extra)
            nc.vector.reciprocal(out=rden, in_=rden)
            cols = slice(b * KT, (b + 1) * KT)
            att_bt = att_ps_pool.tile([KT, 128], F32, tag="abt")
            nc.tensor.transpose(att_bt, att_all[:, cols], eye_sb)
            att_sb = smalls.tile([KT, 128], F32, tag="asb")
            nc.scalar.activation(
                out=att_sb,
                in_=att_bt,
                func=AF.Copy,
                bias=0.0,
                scale=rden[0:KT, :],
            )
            nc.sync.dma_start(
                out=att[b].rearrange("(k p) -> k p", p=128), in_=att_sb
            )

        # ---- main loop: 1-chunk software pipeline -----------------------
        # Emission per iteration: [g0, prev.mv3, g1, prev bookkeeping
        # (scores->exp, reduces, epilogues), mv0, g2, mv1, g3, mv2] so
        # the previous chunk's ACT-latency tail hides behind this
        # chunk's matmul groups in the in-order PE stream.
        wexp_map = {}
        encb_map = {}
        state = {"dens": [], "prev_dens": None, "pending": []}

        def bookkeeping(pb, pc, sc):
            if pc == 0:
                wexp_map[pb] = wexp_pool.tile(
                    [128, L], BF16, tag="wexp", name=f"wexp_{pb}"
                )
                state["dens"] = []
            wexp = wexp_map[pb]
            state["dens"].append(score_to_wexp(sc, wexp, pb, pc))
            for fn in state["pending"]:
                fn()
            state["pending"] = []
            if pc == 1 and pb > 0:
                batch_epilogue(pb - 1, state["prev_dens"])
            if pb >= BL - 2:
                n_act = 6 if (pb == BL - 1 and pc == 1) else 4
                chunk_att_tail(pb, pc, encb_map[pb], wexp, n_act=n_act)
            elif pc == 1:
                state["pending"].append(batch_att(pb, encb_map[pb], wexp))
            if pc == 1:
                state["prev_dens"] = state["dens"]

        prev_bk = (0, 0, sc00)
        prev_mv3 = None
        for b in range(BL):
            for c in range(NLC):
                if (b, c) == (0, 0):
                    continue
                enc_tile = alloc_chunk8(f"enc_{b}_{c}")
                load_chunk8(b, c, enc_tile)
                # bf16-copy prefetch: ~2MB per chunk slot, pulled in
                # for the last two batches (their per-chunk tails read
                # all k-tiles one chunk earlier than batch-wide reduces)
                def _encb(bb, parts, alloc=False):
                    if alloc:
                        encb_map[bb] = alloc_encb(f"encb_{bb}")
                    load_encb_parts(bb, encb_map[bb], parts)

                sched = {
                    (0, 1): [(0, [0, 1], True)],
                    (1, 0): [(0, [2, 3]), (1, [0], True)],
                    (1, 1): [(1, [1, 2])],
                    (2, 0): [(1, [3]), (2, [0], True)],
                    (2, 1): [(2, [1, 2])],
                    (3, 0): [(2, [3]), (3, [0], True)],
                    (3, 1): [(3, [1, 2])],
                    (4, 0): [(3, [3]), (4, [0], True)],
                    (4, 1): [(4, [1, 2])],
                    (5, 0): [(4, [3]), (5, [0, 1], True)],
                    (5, 1): [(5, [2, 3])],
                    (6, 0): [(6, [0, 1], True)],
                    (6, 1): [(6, [2, 3]), (7, [0], True)],
                    (7, 0): [(7, [1, 2, 3])],
                }
                for item in sched.get((b, c), []):
                    _encb(item[0], item[1], len(item) > 2 and item[2])
                sc, groups, mvs = make_chunk(b, c, enc_tile)
                groups[0]()
                if prev_mv3 is not None:
                    prev_mv3()
                groups[1]()
                bookkeeping(*prev_bk)
                mvs[0]()
                groups[2]()
                mvs[1]()
                groups[3]()
                mvs[2]()
                prev_bk = (b, c, sc)
                prev_mv3 = mvs[3]
        prev_mv3()
        bookkeeping(*prev_bk)
        for fn in state["pending"]:
            fn()
        batch_epilogue(BL - 1, state["prev_dens"])

    return nc


def shard_inputs(dec_hidden, enc_output, W_score, b_score, v):
    """Full inputs -> per-core input maps (host-side layout staging)."""
    dec_hidden = np.ascontiguousarray(dec_hidden, dtype=np.float32)
    W_score = np.asarray(W_score, dtype=np.float32)
    # dec_proj + b_score computed host-side (0.05% of the FLOPs)
    bias_full = dec_hidden @ W_score[:DEC] + np.asarray(b_score, dtype=np.float32)
    # W_enc tiled partition-major: [p, k, d]
    wenc8 = np.ascontiguousarray(
        (W_score[DEC:] * WSCALE).reshape(KT, 128, DEC).transpose(1, 0, 2)
    ).astype(ml_dtypes.float8_e4m3)
    v_pd = np.asarray(v, dtype=np.float32).reshape(DT, 128).T * VSCALE
    v_mat = np.zeros((128, DT, 16), dtype=ml_dtypes.float8_e4m3)
    v_mat[:, :, 0] = v_pd.astype(ml_dtypes.float8_e4m3)
    eye = np.eye(128, dtype=np.float32)

    in_maps = []
    for core in range(N_CORES):
        sl = slice(core * BL, (core + 1) * BL)
        # (L, BL, 2E) -> (BL, 2E, L)
        enc_t = np.ascontiguousarray(
            np.asarray(enc_output[:, sl, :], dtype=np.float32).transpose(1, 2, 0)
        )
        # fp8 chunk-major partition-tiled: [b, c, half, p, k, l]
        enc8_t = np.ascontiguousarray(
            enc_t.reshape(BL, 2, KH, 128, NLC, LC).transpose(0, 4, 1, 3, 2, 5)
        ).astype(ml_dtypes.float8_e4m3)
        # bf16 full-L partition-tiled: [b, half, p, k, l]
        encb_t = np.ascontiguousarray(
            enc_t.reshape(BL, 2, KH, 128, L).transpose(0, 1, 3, 2, 4)
        ).astype(ml_dtypes.bfloat16)
        # (BL, DEC) -> [p, dt, b]
        bias_kpb = np.ascontiguousarray(
            bias_full[sl].T.reshape(DT, 128, BL).transpose(1, 0, 2)
        )
        in_maps.append(
            {
                "enc8_t": enc8_t,
                "encb_t": encb_t,
                "ones": np.ones((1, 128), dtype=ml_dtypes.bfloat16),
                "bias_kpb": bias_kpb,
                "wenc8": wenc8,
                "v_mat": v_mat,
                "eye": eye,
            }
        )
    return in_maps


_NC_CACHE = None


def kernel(dec_hidden, enc_output, W_score, b_score, v):
    global _NC_CACHE
    if _NC_CACHE is None:
        _NC_CACHE = build_nc()
    nc = _NC_CACHE
    in_maps = shard_inputs(dec_hidden, enc_output, W_score, b_score, v)
    res = run_bass_kernel_spmd(nc, in_maps, list(range(N_CORES)))
    return np.concatenate([res.results[i]["att"] for i in range(N_CORES)], axis=0)
